# revision 4
# baseline (speedup 1.0000x reference)
"""Trainium2 Bass kernel for nn_MHA_9603546874182 (v2).

Causal MHA: qkv proj + rope(32) + causal attention + out proj.
B=4, T=1024, C=2048, H=32, hd=64.

Sharding: 8-way tensor parallel over heads (4 heads / core).
Host sums the 8 row-parallel output partials (+ bias).

v2 changes vs baseline:
  - bf16 matmul operands (same PE rate, half ldweights cost)
  - compound matmuls (multi-tile moving AP -> 1 LDWEIGHTS + N MATMULs)
  - 256-col causal q-blocking, st-outer loops (fewer wasted rows)
  - softmax denom via exp(-ln(s)) on ACT, batched per (b,h)
  - elementwise work spread across DVE/ACT/Pool engines
"""

import numpy as np

B, T, C, H = 4, 1024, 2048, 32
HD = C // H          # 64
NCORES = 8
HPC = H // NCORES    # 4 heads per core
SC = HPC * HD        # 256 shard channels
NTOK = B * T         # 4096
KT16 = C // 128      # 16 k tiles
MT = NTOK // 128     # 32 token tiles
MPB = T // 128       # 8 token tiles per batch
ROT = 32
NEG = -1.0e9

_CACHE = {}


def _build_nc():
    import concourse.bass as bass
    import concourse.mybir as mybir
    import concourse.tile as tile
    from concourse import bacc
    from concourse.masks import make_identity

    f32 = mybir.dt.float32
    bf16 = mybir.dt.bfloat16
    AF = mybir.ActivationFunctionType

    nc = bacc.Bacc("TRN2")

    xt_d = nc.dram_tensor("xt", [128, KT16, MT, 128], bf16, kind="ExternalInput")
    wq_d = nc.dram_tensor("wq", [128, KT16, 3, SC], bf16, kind="ExternalInput")
    br_d = nc.dram_tensor("br", [128, 2, SC], f32, kind="ExternalInput")
    bv_d = nc.dram_tensor("bv", [128, HPC, HD], f32, kind="ExternalInput")
    c1_d = nc.dram_tensor("c1", [128, MPB, 2, HPC, HD], f32, kind="ExternalInput")
    c2_d = nc.dram_tensor("c2", [128, MPB, 2, HPC, ROT], f32, kind="ExternalInput")
    mk_d = nc.dram_tensor("mk", [128, 384], bf16, kind="ExternalInput")
    w2_d = nc.dram_tensor("w2", [128, 2, C], bf16, kind="ExternalInput")
    out_d = nc.dram_tensor("out", [MT, 128, C], f32, kind="ExternalOutput")

    with tile.TileContext(nc) as tc:
        with (
            tc.tile_pool(name="const", bufs=1) as const,
            tc.tile_pool(name="xp", bufs=3) as xp,
            tc.tile_pool(name="qkp", bufs=3) as qkp,
            tc.tile_pool(name="rtp", bufs=2) as rtp,
            tc.tile_pool(name="bigp", bufs=2) as bigp,
            tc.tile_pool(name="ptp", bufs=3) as ptp,
            tc.tile_pool(name="smp", bufs=2) as smp,
            tc.tile_pool(name="bcp", bufs=2) as bcp,
            tc.tile_pool(name="outp", bufs=3) as outp,
            tc.tile_pool(name="psQ", bufs=2, space="PSUM") as psQ,
            tc.tile_pool(name="psv", bufs=1, space="PSUM") as psv,
            tc.tile_pool(name="pst", bufs=1, space="PSUM") as pst,
            tc.tile_pool(name="psS", bufs=2, space="PSUM") as psS,
            tc.tile_pool(name="psV", bufs=1, space="PSUM") as psV,
        ):
            # weights first (first matmul needs wq); spread queues; w2 last
            identb = const.tile([128, 128], bf16)
            make_identity(nc, identb)
            wq = const.tile([128, KT16, 3, SC], bf16)
            nc.sync.dma_start(wq[:], wq_d[:])
            c1 = const.tile([128, MPB, 2, HPC, HD], f32)
            nc.scalar.dma_start(c1[:], c1_d[:])
            c2 = const.tile([128, MPB, 2, HPC, ROT], f32)
            nc.scalar.dma_start(c2[:], c2_d[:])
            br = const.tile([128, 2, SC], f32)
            nc.gpsimd.dma_start(br[:], br_d[:])
            bv = const.tile([128, HPC, HD], f32)
            nc.gpsimd.dma_start(bv[:], bv_d[:])
            mk = const.tile([128, 384], bf16)
            nc.gpsimd.dma_start(mk[:], mk_d[:])
            w2 = const.tile([128, 2, C], bf16)
            nc.gpsimd.dma_start(w2[:], w2_d[:])

            for b in range(B):
                # QKT layout: [128, 4, T] slots: 0=q(ch0:128) 1=q(ch128:256)
                #                                2=k(ch0:128) 3=k(ch128:256)
                QKT = bigp.tile([128, 4, T], bf16, tag="qkt")
                Vp = bigp.tile([128, MPB, HPC, HD + 1], bf16, tag="vp")
                ctxT = bigp.tile([128, 2, T], bf16, tag="ct")
                nc.gpsimd.memset(Vp[:, :, :, HD:HD + 1], 1.0)

                # ---- phase 1: qkv + rope + transpose ----
                for m8 in range(MPB):
                    m = b * MPB + m8
                    xt = xp.tile([128, KT16, 128], bf16)
                    nc.sync.dma_start(xt[:], xt_d[:, :, m, :])
                    psq = psQ.tile([128, 2, SC], f32, tag="qk")
                    psvv = psv.tile([128, SC], f32, tag="v")
                    for k in range(KT16):
                        nc.tensor.matmul(
                            psq[:], xt[:, k, :], wq[:, k, 0:2, :],
                            start=(k == 0), stop=(k == KT16 - 1))
                        nc.tensor.matmul(
                            psvv[:], xt[:, k, :], wq[:, k, 2, :],
                            start=(k == 0), stop=(k == KT16 - 1))
                    # v: bias add straight into Vp (token-major)
                    nc.vector.tensor_add(
                        Vp[:, m8, :, 0:HD],
                        psvv.rearrange("p (h d) -> p h d", h=HPC),
                        bv[:])
                    # q/k: bias add then rope (DVE), bf16 out
                    psq4 = psq.rearrange("p s (h d) -> p s h d", h=HPC)
                    nc.vector.tensor_add(psq[:], psq[:], br[:])
                    rt = rtp.tile([128, 2, HPC, ROT], f32)
                    nc.vector.tensor_mul(
                        rt[:, :, :, 0:16], psq4[:, :, :, 16:32],
                        c2[:, m8, :, :, 0:16])
                    nc.vector.tensor_mul(
                        rt[:, :, :, 16:32], psq4[:, :, :, 0:16],
                        c2[:, m8, :, :, 16:32])
                    qkb = qkp.tile([128, 512], bf16)
                    qkb4 = qkb.rearrange("p (s h d) -> p s h d", s=2, h=HPC)
                    nc.vector.tensor_mul(qkb4[:], psq4[:], c1[:, m8])
                    nc.vector.tensor_add(
                        qkb4[:, :, :, 0:ROT], qkb4[:, :, :, 0:ROT], rt[:])
                    # transpose q/k -> QKT (bf16 PE transposes into one psum)
                    tp = pst.tile([128, 4, 128], bf16, tag="tp")
                    for ci in range(4):
                        nc.tensor.transpose(
                            tp[:, ci, :], qkb[:, ci * 128:(ci + 1) * 128],
                            identb)
                    nc.vector.tensor_copy(
                        QKT[:, :, m8 * 128:(m8 + 1) * 128], tp[:])

                # ---- phase 2: attention (256-col q groups, st-outer) ----
                for h in range(HPC):
                    p0 = (h % 2) * 64
                    qt_h = QKT[p0:p0 + 64, h // 2, :].rearrange(
                        "p (g x) -> p g x", g=4)
                    kt_h = QKT[p0:p0 + 64, 2 + h // 2, :]
                    # av split into two 1-bank psum tiles (groups 0-1, 2-3)
                    av01 = psV.tile([HD + 1, 2, 256], f32, tag="av0")
                    av23 = psV.tile([HD + 1, 2, 256], f32, tag="av1")
                    avt = {0: av01, 1: av23}

                    def av_mm(ga, gb, v_st, pt, st):
                        # stop only on the final write to each av bank
                        t = avt[ga // 2]
                        stop = (st == 3 and ga < 2) or st == MPB - 1
                        nc.tensor.matmul(
                            t[:, ga - (ga // 2) * 2:gb - (ga // 2) * 2, :],
                            v_st, pt[:, ga:gb, :], start=(st == 0), stop=stop)

                    for st in range(MPB):
                        g0 = st // 2
                        kt_st = kt_h[:, st * 128:(st + 1) * 128]
                        pt = ptp.tile([128, 4, 256], bf16)
                        # scores + mask + exp, in <=2-group chunks
                        ga = g0
                        while ga < 4:
                            gb = 2 if ga < 2 else 4
                            sc = psS.tile([128, 2, 256], f32, tag="sc")
                            nc.tensor.matmul(
                                sc[:, 0:gb - ga, :], kt_st, qt_h[:, ga:gb, :],
                                start=True, stop=True)
                            nc.scalar.activation(
                                pt[:, ga:gb, :], sc[:, 0:gb - ga, :], AF.Exp)
                            ga = gb
                        # causal mask: zero future probs (0/1 mul, Pool)
                        if st % 2 == 0:
                            nc.gpsimd.tensor_mul(
                                pt[:, g0, 0:128], pt[:, g0, 0:128],
                                mk[:, 0:128])
                        else:
                            nc.gpsimd.tensor_mul(
                                pt[:, g0, :], pt[:, g0, :], mk[:, 128:384])
                        # AV accumulate (ones column in Vp gives col sums)
                        v_st = Vp[:, st, h, :]
                        ga = g0
                        while ga < 4:
                            gb = 2 if ga < 2 else 4
                            av_mm(ga, gb, v_st, pt, st)
                            ga = gb
                    # normalize: inv = exp(-ln(sum)); ctxT = av * inv
                    sums = smp.tile([1, 1024], f32, tag="sums")
                    invs = smp.tile([1, 1024], f32, tag="invs")
                    nc.vector.tensor_copy(
                        sums[:, 0:512],
                        av01[HD:HD + 1, :, :].rearrange("p g x -> p (g x)"))
                    nc.vector.tensor_copy(
                        sums[:, 512:1024],
                        av23[HD:HD + 1, :, :].rearrange("p g x -> p (g x)"))
                    bci = bcp.tile([64, 1024], f32)
                    nc.vector.reciprocal_approx_fast(invs[:], sums[:])
                    nc.gpsimd.partition_broadcast(bci[:], invs[:])
                    nc.vector.tensor_mul(
                        ctxT[p0:p0 + 64, h // 2, 0:512],
                        av01[0:HD, :, :].rearrange("p g x -> p (g x)"),
                        bci[:, 0:512])
                    nc.vector.tensor_mul(
                        ctxT[p0:p0 + 64, h // 2, 512:1024],
                        av23[0:HD, :, :].rearrange("p g x -> p (g x)"),
                        bci[:, 512:1024])

                # ---- phase 3: out projection partial ----
                for m8 in range(MPB):
                    m = b * MPB + m8
                    ot = outp.tile([128, C], f32)
                    for n in range(4):
                        po_raw = psS.tile([128, 2, 256], f32, tag="sc")
                        po = po_raw.rearrange("p s j -> p (s j)")
                        for j in range(2):
                            nc.tensor.matmul(
                                po[:],
                                ctxT[:, j, m8 * 128:(m8 + 1) * 128],
                                w2[:, j, n * 512:(n + 1) * 512],
                                start=(j == 0), stop=(j == 1))
                        if n % 2 == 0:
                            nc.scalar.copy(ot[:, n * 512:(n + 1) * 512], po[:])
                        else:
                            nc.vector.tensor_copy(
                                ot[:, n * 512:(n + 1) * 512], po[:])
                    nc.gpsimd.dma_start(out_d[m, :, :], ot[:])

    nc.finalize()
    return nc


def _host_prep(x, rope, Wqkv_w, Wqkv_b, out_w):
    """Build per-core input maps (partition-first layouts, bf16 operands)."""
    import ml_dtypes
    bf16 = ml_dtypes.bfloat16

    xf = np.ascontiguousarray(x.reshape(NTOK, C)).astype(np.float32)
    # xt[p, k, m, t] = x[m*128+t, k*128+p]
    xt = np.ascontiguousarray(
        xf.reshape(MT, 128, KT16, 128).transpose(3, 2, 0, 1)).astype(bf16)

    # rope tables (position within a batch: t = 0..1023)
    cos = rope[:, :, 0].astype(np.float32)   # [T, 16]
    sin = rope[:, :, 1].astype(np.float32)
    C1h = np.ones((T, HD), np.float32)
    C1h[:, 0:16] = cos
    C1h[:, 16:32] = cos
    C2h = np.zeros((T, ROT), np.float32)
    C2h[:, 0:16] = -sin
    C2h[:, 16:32] = sin
    # c1[p, m8, s, h, d] = C1h[m8*128+p, d]
    c1 = np.ascontiguousarray(np.broadcast_to(
        C1h.reshape(MPB, 128, 1, 1, HD).transpose(1, 0, 2, 3, 4),
        (128, MPB, 2, HPC, HD))).astype(np.float32)
    c2 = np.ascontiguousarray(np.broadcast_to(
        C2h.reshape(MPB, 128, 1, 1, ROT).transpose(1, 0, 2, 3, 4),
        (128, MPB, 2, HPC, ROT))).astype(np.float32)

    # causal keep-mask (0/1) for diagonal 128x256 score^T tiles
    # A[p, c] (cols 0:128):  0 if c < p       (st = 2g)
    # B[p, c] (cols 128:384): 0 if c < p+128  (st = 2g+1)
    pp = np.arange(128)[:, None]
    cA = np.arange(128)[None, :]
    cB = np.arange(256)[None, :]
    mk = np.concatenate([
        np.where(cA < pp, np.float32(0.0), np.float32(1.0)),
        np.where(cB < pp + 128, np.float32(0.0), np.float32(1.0)),
    ], axis=1).astype(bf16)

    scale = np.float32(1.0 / np.sqrt(HD))
    in_maps = []
    for g in range(NCORES):
        hs = g * SC
        Wq = Wqkv_w[hs:hs + SC, :].astype(np.float32) * scale
        Wk = Wqkv_w[C + hs:C + hs + SC, :].astype(np.float32)
        Wv = Wqkv_w[2 * C + hs:2 * C + hs + SC, :].astype(np.float32)
        Wsh = np.concatenate([Wq, Wk, Wv], axis=0)          # [768, 2048]
        # wq[p, k, s, j] = Wsh[s*256+j, k*128+p]
        wqa = np.ascontiguousarray(
            Wsh.T.reshape(KT16, 128, 3, SC).transpose(1, 0, 2, 3)).astype(bf16)
        bq = Wqkv_b[hs:hs + SC].astype(np.float32) * scale
        bk = Wqkv_b[C + hs:C + hs + SC].astype(np.float32)
        bvv = Wqkv_b[2 * C + hs:2 * C + hs + SC].astype(np.float32)
        bra = np.ascontiguousarray(np.broadcast_to(
            np.stack([bq, bk]), (128, 2, SC))).astype(np.float32)
        bva = np.ascontiguousarray(np.broadcast_to(
            bvv.reshape(HPC, HD), (128, HPC, HD))).astype(np.float32)
        # w2[p, j, o] = out_w[o, g*256 + j*128 + p]
        w2a = np.ascontiguousarray(
            out_w[:, hs:hs + SC].astype(np.float32).T.reshape(
                2, 128, C).transpose(1, 0, 2)).astype(bf16)
        in_maps.append({
            "xt": xt, "wq": wqa, "br": bra, "bv": bva, "c1": c1, "c2": c2,
            "mk": mk, "w2": w2a,
        })
    return in_maps


def kernel(x, mask, index, rope, Wqkv_w, Wqkv_b, out_w, out_b,
           k_cache, v_cache):
    from concourse.bass_utils import run_bass_kernel_spmd

    x = np.asarray(x)
    rope = np.asarray(rope)
    Wqkv_w = np.asarray(Wqkv_w)
    Wqkv_b = np.asarray(Wqkv_b)
    out_w = np.asarray(out_w)
    out_b = np.asarray(out_b)

    if "nc" not in _CACHE:
        _CACHE["nc"] = _build_nc()
    nc = _CACHE["nc"]

    in_maps = _host_prep(x, rope, Wqkv_w, Wqkv_b, out_w)
    res = run_bass_kernel_spmd(nc, in_maps, core_ids=list(range(NCORES)))

    acc = np.zeros((NTOK, C), np.float32)
    for g in range(NCORES):
        acc += res.results[g]["out"].reshape(NTOK, C)
    acc += out_b.astype(np.float32)
    return acc.reshape(B, T, C)


# revision 5
# speedup vs baseline: 1.5600x; 1.5600x over previous
"""Trainium2 Bass kernel for nn_MHA_9603546874182 (v2).

Causal MHA: qkv proj + rope(32) + causal attention + out proj.
B=4, T=1024, C=2048, H=32, hd=64.

Sharding: 8-way tensor parallel over heads (4 heads / core).
Host sums the 8 row-parallel output partials (+ bias).

v2 changes vs baseline:
  - bf16 matmul operands (same PE rate, half ldweights cost)
  - compound matmuls (multi-tile moving AP -> 1 LDWEIGHTS + N MATMULs)
  - 256-col causal q-blocking, st-outer loops (fewer wasted rows)
  - softmax denom via exp(-ln(s)) on ACT, batched per (b,h)
  - elementwise work spread across DVE/ACT/Pool engines
"""

import numpy as np

B, T, C, H = 4, 1024, 2048, 32
HD = C // H          # 64
NCORES = 8
HPC = H // NCORES    # 4 heads per core
SC = HPC * HD        # 256 shard channels
NTOK = B * T         # 4096
KT16 = C // 128      # 16 k tiles
MT = NTOK // 128     # 32 token tiles
MPB = T // 128       # 8 token tiles per batch
ROT = 32
NEG = -1.0e9

_CACHE = {}


def _build_nc():
    import concourse.bass as bass
    import concourse.mybir as mybir
    import concourse.tile as tile
    from concourse import bacc
    from concourse.masks import make_identity

    f32 = mybir.dt.float32
    bf16 = mybir.dt.bfloat16
    AF = mybir.ActivationFunctionType

    nc = bacc.Bacc("TRN2")

    xt_d = nc.dram_tensor("xt", [128, KT16, MT, 128], bf16, kind="ExternalInput")
    wq_d = nc.dram_tensor("wq", [128, KT16, 3, SC], bf16, kind="ExternalInput")
    br_d = nc.dram_tensor("br", [128, 2, SC], f32, kind="ExternalInput")
    bv_d = nc.dram_tensor("bv", [128, HPC, HD], f32, kind="ExternalInput")
    c1_d = nc.dram_tensor("c1", [128, MPB, HD], f32, kind="ExternalInput")
    c2_d = nc.dram_tensor("c2", [128, MPB, ROT], f32, kind="ExternalInput")
    mk_d = nc.dram_tensor("mk", [128, 384], bf16, kind="ExternalInput")
    w2_d = nc.dram_tensor("w2", [128, 2, C], bf16, kind="ExternalInput")
    out_d = nc.dram_tensor("out", [MT, 128, C], f32, kind="ExternalOutput")

    with tile.TileContext(nc) as tc:
        with (
            tc.tile_pool(name="const", bufs=1) as const,
            tc.tile_pool(name="xp", bufs=3) as xp,
            tc.tile_pool(name="qkp", bufs=3) as qkp,
            tc.tile_pool(name="rtp", bufs=2) as rtp,
            tc.tile_pool(name="bigp", bufs=2) as bigp,
            tc.tile_pool(name="ptp", bufs=3) as ptp,
            tc.tile_pool(name="smp", bufs=2) as smp,
            tc.tile_pool(name="bcp", bufs=2) as bcp,
            tc.tile_pool(name="outp", bufs=3) as outp,
            tc.tile_pool(name="psQ", bufs=2, space="PSUM") as psQ,
            tc.tile_pool(name="psv", bufs=1, space="PSUM") as psv,
            tc.tile_pool(name="pst", bufs=1, space="PSUM") as pst,
            tc.tile_pool(name="psS", bufs=2, space="PSUM") as psS,
            tc.tile_pool(name="psV", bufs=1, space="PSUM") as psV,
        ):
            # weights first (first matmul needs wq); spread queues; w2 last
            identb = const.tile([128, 128], bf16)
            make_identity(nc, identb)
            wq = const.tile([128, KT16, 3, SC], bf16)
            nc.sync.dma_start(wq[:], wq_d[:])
            c1 = const.tile([128, MPB, HD], f32)
            nc.scalar.dma_start(c1[:], c1_d[:])
            c2 = const.tile([128, MPB, ROT], f32)
            nc.scalar.dma_start(c2[:], c2_d[:])
            br = const.tile([128, 2, SC], f32)
            nc.gpsimd.dma_start(br[:], br_d[:])
            bv = const.tile([128, HPC, HD], f32)
            nc.gpsimd.dma_start(bv[:], bv_d[:])
            mk = const.tile([128, 384], bf16)
            nc.gpsimd.dma_start(mk[:], mk_d[:])
            w2 = const.tile([128, 2, C], bf16)
            nc.gpsimd.dma_start(w2[:], w2_d[:])

            for b in range(B):
                # QKT layout: [128, 4, T] slots: 0=q(ch0:128) 1=q(ch128:256)
                #                                2=k(ch0:128) 3=k(ch128:256)
                QKT = bigp.tile([128, 4, T], bf16, tag="qkt")
                Vp = bigp.tile([128, MPB, HPC, HD + 1], bf16, tag="vp")
                ctxT = bigp.tile([128, 2, T], bf16, tag="ct")
                nc.gpsimd.memset(Vp[:, :, :, HD:HD + 1], 1.0)

                # ---- phase 1: qkv + rope + transpose ----
                for m8 in range(MPB):
                    m = b * MPB + m8
                    xt = xp.tile([128, KT16, 128], bf16)
                    nc.sync.dma_start(xt[:], xt_d[:, :, m, :])
                    psq = psQ.tile([128, 2, SC], f32, tag="qk")
                    psvv = psv.tile([128, SC], f32, tag="v")
                    for k in range(KT16):
                        nc.tensor.matmul(
                            psq[:], xt[:, k, :], wq[:, k, 0:2, :],
                            start=(k == 0), stop=(k == KT16 - 1))
                        nc.tensor.matmul(
                            psvv[:], xt[:, k, :], wq[:, k, 2, :],
                            start=(k == 0), stop=(k == KT16 - 1))
                    # v: bias add straight into Vp (token-major)
                    nc.vector.tensor_add(
                        Vp[:, m8, :, 0:HD],
                        psvv.rearrange("p (h d) -> p h d", h=HPC),
                        bv[:])
                    # q/k: bias add then rope (DVE), bf16 out
                    psq4 = psq.rearrange("p s (h d) -> p s h d", h=HPC)
                    nc.vector.tensor_add(psq[:], psq[:], br[:])
                    rt = rtp.tile([128, 2, HPC, ROT], f32)
                    c2a = c2[:, m8, 0:16].unsqueeze(1).unsqueeze(1) \
                        .broadcast_to([128, 2, HPC, 16])
                    c2b = c2[:, m8, 16:32].unsqueeze(1).unsqueeze(1) \
                        .broadcast_to([128, 2, HPC, 16])
                    nc.vector.tensor_mul(
                        rt[:, :, :, 0:16], psq4[:, :, :, 16:32], c2a)
                    nc.vector.tensor_mul(
                        rt[:, :, :, 16:32], psq4[:, :, :, 0:16], c2b)
                    qkb = qkp.tile([128, 512], bf16)
                    qkb4 = qkb.rearrange("p (s h d) -> p s h d", s=2, h=HPC)
                    c1b = c1[:, m8].unsqueeze(1).unsqueeze(1) \
                        .broadcast_to([128, 2, HPC, HD])
                    nc.vector.tensor_mul(qkb4[:], psq4[:], c1b)
                    nc.vector.tensor_add(
                        qkb4[:, :, :, 0:ROT], qkb4[:, :, :, 0:ROT], rt[:])
                    # transpose q/k -> QKT (bf16 PE transposes into one psum)
                    tp = pst.tile([128, 4, 128], bf16, tag="tp")
                    for ci in range(4):
                        nc.tensor.transpose(
                            tp[:, ci, :], qkb[:, ci * 128:(ci + 1) * 128],
                            identb)
                    nc.vector.tensor_copy(
                        QKT[:, :, m8 * 128:(m8 + 1) * 128], tp[:])

                # ---- phase 2: attention (256-col q groups, st-outer) ----
                for h in range(HPC):
                    p0 = (h % 2) * 64
                    qt_h = QKT[p0:p0 + 64, h // 2, :].rearrange(
                        "p (g x) -> p g x", g=4)
                    kt_h = QKT[p0:p0 + 64, 2 + h // 2, :]
                    # av split into two 1-bank psum tiles (groups 0-1, 2-3)
                    av01 = psV.tile([HD + 1, 2, 256], f32, tag="av0")
                    av23 = psV.tile([HD + 1, 2, 256], f32, tag="av1")
                    avt = {0: av01, 1: av23}

                    def av_mm(ga, gb, v_st, pt, st):
                        # stop only on the final write to each av bank
                        t = avt[ga // 2]
                        stop = (st == 3 and ga < 2) or st == MPB - 1
                        nc.tensor.matmul(
                            t[:, ga - (ga // 2) * 2:gb - (ga // 2) * 2, :],
                            v_st, pt[:, ga:gb, :], start=(st == 0), stop=stop)

                    for st in range(MPB):
                        g0 = st // 2
                        kt_st = kt_h[:, st * 128:(st + 1) * 128]
                        pt = ptp.tile([128, 4, 256], bf16)
                        # scores + mask + exp, in <=2-group chunks
                        ga = g0
                        while ga < 4:
                            gb = 2 if ga < 2 else 4
                            sc = psS.tile([128, 2, 256], f32, tag="sc")
                            nc.tensor.matmul(
                                sc[:, 0:gb - ga, :], kt_st, qt_h[:, ga:gb, :],
                                start=True, stop=True)
                            nc.scalar.activation(
                                pt[:, ga:gb, :], sc[:, 0:gb - ga, :], AF.Exp)
                            ga = gb
                        # causal mask: zero future probs (bf16 DVE mul)
                        if st % 2 == 0:
                            nc.vector.tensor_mul(
                                pt[:, g0, 0:128], pt[:, g0, 0:128],
                                mk[:, 0:128])
                        else:
                            nc.vector.tensor_mul(
                                pt[:, g0, :], pt[:, g0, :], mk[:, 128:384])
                        # AV accumulate (ones column in Vp gives col sums)
                        v_st = Vp[:, st, h, :]
                        ga = g0
                        while ga < 4:
                            gb = 2 if ga < 2 else 4
                            av_mm(ga, gb, v_st, pt, st)
                            ga = gb
                    # normalize: inv = exp(-ln(sum)); ctxT = av * inv
                    sums = smp.tile([1, 1024], f32, tag="sums")
                    invs = smp.tile([1, 1024], f32, tag="invs")
                    nc.vector.tensor_copy(
                        sums[:, 0:512],
                        av01[HD:HD + 1, :, :].rearrange("p g x -> p (g x)"))
                    nc.vector.tensor_copy(
                        sums[:, 512:1024],
                        av23[HD:HD + 1, :, :].rearrange("p g x -> p (g x)"))
                    bci = bcp.tile([64, 1024], f32)
                    nc.vector.reciprocal_approx_fast(invs[:], sums[:])
                    nc.gpsimd.partition_broadcast(bci[:], invs[:])
                    nc.vector.tensor_mul(
                        ctxT[p0:p0 + 64, h // 2, 0:512],
                        av01[0:HD, :, :].rearrange("p g x -> p (g x)"),
                        bci[:, 0:512])
                    nc.vector.tensor_mul(
                        ctxT[p0:p0 + 64, h // 2, 512:1024],
                        av23[0:HD, :, :].rearrange("p g x -> p (g x)"),
                        bci[:, 512:1024])

                # ---- phase 3: out projection partial ----
                for m8 in range(MPB):
                    m = b * MPB + m8
                    ot = outp.tile([128, C], f32)
                    for n in range(4):
                        po_raw = psS.tile([128, 2, 256], f32, tag="sc")
                        po = po_raw.rearrange("p s j -> p (s j)")
                        for j in range(2):
                            nc.tensor.matmul(
                                po[:],
                                ctxT[:, j, m8 * 128:(m8 + 1) * 128],
                                w2[:, j, n * 512:(n + 1) * 512],
                                start=(j == 0), stop=(j == 1))
                        if n % 2 == 0:
                            nc.scalar.copy(ot[:, n * 512:(n + 1) * 512], po[:])
                        else:
                            nc.vector.tensor_copy(
                                ot[:, n * 512:(n + 1) * 512], po[:])
                    nc.gpsimd.dma_start(out_d[m, :, :], ot[:])

    nc.finalize()
    return nc


def _host_prep(x, rope, Wqkv_w, Wqkv_b, out_w):
    """Build per-core input maps (partition-first layouts, bf16 operands)."""
    import ml_dtypes
    bf16 = ml_dtypes.bfloat16

    xf = np.ascontiguousarray(x.reshape(NTOK, C)).astype(np.float32)
    # xt[p, k, m, t] = x[m*128+t, k*128+p]
    xt = np.ascontiguousarray(
        xf.reshape(MT, 128, KT16, 128).transpose(3, 2, 0, 1)).astype(bf16)

    # rope tables (position within a batch: t = 0..1023)
    cos = rope[:, :, 0].astype(np.float32)   # [T, 16]
    sin = rope[:, :, 1].astype(np.float32)
    C1h = np.ones((T, HD), np.float32)
    C1h[:, 0:16] = cos
    C1h[:, 16:32] = cos
    C2h = np.zeros((T, ROT), np.float32)
    C2h[:, 0:16] = -sin
    C2h[:, 16:32] = sin
    # c1[p, m8, d] = C1h[m8*128+p, d]
    c1 = np.ascontiguousarray(
        C1h.reshape(MPB, 128, HD).transpose(1, 0, 2)).astype(np.float32)
    c2 = np.ascontiguousarray(
        C2h.reshape(MPB, 128, ROT).transpose(1, 0, 2)).astype(np.float32)

    # causal keep-mask (0/1) for diagonal 128x256 score^T tiles
    # A[p, c] (cols 0:128):  0 if c < p       (st = 2g)
    # B[p, c] (cols 128:384): 0 if c < p+128  (st = 2g+1)
    pp = np.arange(128)[:, None]
    cA = np.arange(128)[None, :]
    cB = np.arange(256)[None, :]
    mk = np.concatenate([
        np.where(cA < pp, np.float32(0.0), np.float32(1.0)),
        np.where(cB < pp + 128, np.float32(0.0), np.float32(1.0)),
    ], axis=1).astype(bf16)

    scale = np.float32(1.0 / np.sqrt(HD))
    in_maps = []
    for g in range(NCORES):
        hs = g * SC
        Wq = Wqkv_w[hs:hs + SC, :].astype(np.float32) * scale
        Wk = Wqkv_w[C + hs:C + hs + SC, :].astype(np.float32)
        Wv = Wqkv_w[2 * C + hs:2 * C + hs + SC, :].astype(np.float32)
        Wsh = np.concatenate([Wq, Wk, Wv], axis=0)          # [768, 2048]
        # wq[p, k, s, j] = Wsh[s*256+j, k*128+p]
        wqa = np.ascontiguousarray(
            Wsh.T.reshape(KT16, 128, 3, SC).transpose(1, 0, 2, 3)).astype(bf16)
        bq = Wqkv_b[hs:hs + SC].astype(np.float32) * scale
        bk = Wqkv_b[C + hs:C + hs + SC].astype(np.float32)
        bvv = Wqkv_b[2 * C + hs:2 * C + hs + SC].astype(np.float32)
        bra = np.ascontiguousarray(np.broadcast_to(
            np.stack([bq, bk]), (128, 2, SC))).astype(np.float32)
        bva = np.ascontiguousarray(np.broadcast_to(
            bvv.reshape(HPC, HD), (128, HPC, HD))).astype(np.float32)
        # w2[p, j, o] = out_w[o, g*256 + j*128 + p]
        w2a = np.ascontiguousarray(
            out_w[:, hs:hs + SC].astype(np.float32).T.reshape(
                2, 128, C).transpose(1, 0, 2)).astype(bf16)
        in_maps.append({
            "xt": xt, "wq": wqa, "br": bra, "bv": bva, "c1": c1, "c2": c2,
            "mk": mk, "w2": w2a,
        })
    return in_maps


def kernel(x, mask, index, rope, Wqkv_w, Wqkv_b, out_w, out_b,
           k_cache, v_cache):
    from concourse.bass_utils import run_bass_kernel_spmd

    x = np.asarray(x)
    rope = np.asarray(rope)
    Wqkv_w = np.asarray(Wqkv_w)
    Wqkv_b = np.asarray(Wqkv_b)
    out_w = np.asarray(out_w)
    out_b = np.asarray(out_b)

    if "nc" not in _CACHE:
        _CACHE["nc"] = _build_nc()
    nc = _CACHE["nc"]

    in_maps = _host_prep(x, rope, Wqkv_w, Wqkv_b, out_w)
    res = run_bass_kernel_spmd(nc, in_maps, core_ids=list(range(NCORES)))

    acc = np.zeros((NTOK, C), np.float32)
    for g in range(NCORES):
        acc += res.results[g]["out"].reshape(NTOK, C)
    acc += out_b.astype(np.float32)
    return acc.reshape(B, T, C)
